# revision 8
# baseline (speedup 1.0000x reference)
"""Trainium2 Bass kernel for nn_CustomMultiHead (96 Linear(2048,1) heads).

Computes out[16384, 96] = x[16384, 2048] @ W.T[2048, 96] + b.

Strategy (data-parallel over batch, 8 cores):
  - Host pre-transposes each core's x shard to xT[f, b] so the device
    kernel needs no on-chip transpose (PE matmul contracts along the
    partition dim).
  - Per core: out.T[96, 2048] = sum_k W.T[k] .T-contracted xT[k] tiles.
    lhsT = W.T tile [128, 96] (stationary), rhs = xT tile [128, 512]
    (moving, N=512 hits the float32r full-rate PE path), PSUM [96, 512]
    accumulates over the 16 k-tiles; bias added on the PSUM->SBUF copy.
  - Host transposes/concats the 8 out.T shards back to [16384, 96].
"""

import os

import numpy as np

import concourse.mybir as mybir
import concourse.tile as tile
from concourse import bacc
from concourse.bass_utils import run_bass_kernel_spmd

N_CORES = 8
B_FULL = 16384
F = 2048  # contraction (in_features)
H = 96  # heads
B_CORE = B_FULL // N_CORES  # 2048 batch rows per core
P = 128  # partitions
KT = F // P  # 16 k-tiles
BN = 512  # moving free dim per matmul (one PSUM bank of fp32)
BT = B_CORE // BN  # 4 output column tiles per core

_NC_CACHE = {}


def _build(repeat=1, use_f32r=True, timing_mode=False):
    f32 = mybir.dt.float32
    mm_dt = mybir.dt.float32r if use_f32r else mybir.dt.float32

    nc = bacc.Bacc("TRN2", target_bir_lowering=False, debug=False, num_devices=N_CORES)
    if not timing_mode:
        xT = nc.dram_tensor("xT", [F, B_CORE], mm_dt, kind="ExternalInput")
    wT = nc.dram_tensor("wT", [F, H], mm_dt, kind="ExternalInput")
    bias = nc.dram_tensor("bias", [H, 1], f32, kind="ExternalInput")
    outT = nc.dram_tensor("outT", [H, B_CORE], f32, kind="ExternalOutput")

    with tile.TileContext(nc) as tc:
        if timing_mode:
            # x lives in internal DRAM (garbage contents): identical DMA and
            # compute pattern, but launches don't ship the 16MB/core shard.
            with tc.tile_pool(name="xdram", bufs=1, space="DRAM") as xdram:
                xT = xdram.tile([F, B_CORE], mm_dt, name="xT_int")
        KG = 2  # k-stripes per DMA (2MB transfers)
        with (
            tc.tile_pool(name="wpool", bufs=1) as wpool,
            tc.tile_pool(name="xpool", bufs=3) as xpool,
            tc.tile_pool(name="pspool", bufs=1, space="PSUM") as pspool,
            tc.tile_pool(name="opool", bufs=2) as opool,
        ):
            wt = wpool.tile([P, KT, H], mm_dt)
            nc.sync.dma_start(wt[:], wT.ap().rearrange("(t p) h -> p t h", p=P))
            bias_sb = wpool.tile([H, 1], f32)
            nc.sync.dma_start(bias_sb[:], bias[:])

            def body(_=None):
                psums = [
                    pspool.tile([H, BN], f32, name=f"ps{i}", tag=f"ps{i}")
                    for i in range(BT)
                ]
                for kg in range(KT // KG):
                    xk = xpool.tile([P, KG, B_CORE], mm_dt, tag="xk")
                    # [128, KG, 2048] <- KG adjacent k-stripes in one DMA;
                    # alternate the two HWDGE rings (SP / ACT).
                    dma_eng = nc.sync if kg % 2 == 0 else nc.scalar
                    dma_eng.dma_start(
                        xk[:],
                        xT[kg * KG * P : (kg + 1) * KG * P, :].rearrange(
                            "(s p) b -> p s b", p=P
                        ),
                    )
                    for s in range(KG):
                        k = kg * KG + s
                        for bt in range(BT):
                            nc.tensor.matmul(
                                psums[bt][:],
                                lhsT=wt[:, k, :],
                                rhs=xk[:, s, bt * BN : (bt + 1) * BN],
                                start=(k == 0),
                                stop=(k == KT - 1),
                            )
                for bt in range(BT):
                    ot = opool.tile([H, BN], f32, tag="ot")
                    nc.vector.tensor_scalar_add(ot[:], psums[bt][:], bias_sb[:])
                    nc.sync.dma_start(outT[:, bt * BN : (bt + 1) * BN], ot[:])

            if repeat == 1:
                body()
            else:
                with tc.For_i(0, repeat, 1):
                    body()

    nc.compile()
    return nc


def _get_nc(repeat, use_f32r, timing_mode=False):
    key = (repeat, use_f32r, timing_mode)
    if key not in _NC_CACHE:
        _NC_CACHE[key] = _build(repeat, use_f32r, timing_mode)
    return _NC_CACHE[key]


def kernel(x, W, b):
    repeat = int(os.environ.get("BASS_KERNEL_REPEAT", "1"))
    use_f32r = os.environ.get("BASS_KERNEL_F32R", "1") == "1"
    timing_mode = os.environ.get("BASS_KERNEL_TIMING", "0") == "1"
    nc = _get_nc(repeat, use_f32r, timing_mode)

    x = np.ascontiguousarray(x, dtype=np.float32)
    wT_host = np.ascontiguousarray(W.T, dtype=np.float32)
    bias_host = np.ascontiguousarray(np.asarray(b, dtype=np.float32).reshape(H, 1))

    in_maps = []
    for i in range(N_CORES):
        shard = x[i * B_CORE : (i + 1) * B_CORE, :]
        m = {
            "wT": wT_host,
            "bias": bias_host,
        }
        if not timing_mode:
            m["xT"] = np.ascontiguousarray(shard.T)
        in_maps.append(m)

    res = run_bass_kernel_spmd(nc, in_maps, core_ids=list(range(N_CORES)))
    out = np.concatenate(
        [np.ascontiguousarray(res.results[i]["outT"].T) for i in range(N_CORES)],
        axis=0,
    )
    return out


# revision 49
# speedup vs baseline: 1.6738x; 1.6738x over previous
"""Trainium2 Bass kernel for nn_CustomMultiHead (96 Linear(2048,1) heads).

Computes out[16384, 96] = x[16384, 2048] @ W.T[2048, 96] + b.

Strategy (data-parallel over batch, 8 cores, 2048 rows each):
  - Host pre-transposes each core's x shard into partition-major
    xTp[p, t, b] (p=partition, t=k-tile, b=batch) so the device kernel
    needs no on-chip transpose (PE matmul contracts along the partition
    dim) and every DMA reads one large contiguous run per partition.
  - Inputs are cast to fp16 on the host: halves HBM traffic (the kernel
    is memory-bound) at ~2.8e-4 scale-relative absmax error; PSUM
    accumulation stays fp32.
  - Per core: out.T[96, 2048] accumulated over 16 k-tiles of 128.
    lhsT = W.T tile [128, 96] (stationary), rhs = xTp tile [128, 512]
    (moving, N=512 = one fp32 PSUM bank); bias added on the PSUM->SBUF
    copy (DVE tensor_scalar_add with a per-partition scalar).
  - x streams through SBUF in 2 double-buffered 4MB DMA groups (8
    k-stripes each) overlapping the matmul stream; in the final group
    the matmuls run bt-major so each PSUM's copy/out-DMA overlaps the
    remaining matmuls.
  - Host transposes/concats the 8 out.T shards back to [16384, 96].

Measured (8-core SPMD, axon): ~30-35us/core vs ~28.6us pure-DMA floor
at the observed ~290GB/s effective HBM rate. fp32r variant (exact fp32
inputs, TF32-class PE path): ~55-60us at 1.2e-4 error; plain fp32:
~80us at 4e-7 (PE-bound, 4 cycles/row). Env knobs (BASS_KERNEL_MM,
BASS_KG, ...) select variants; defaults are the shipped configuration.
"""

import os

import numpy as np

import concourse.mybir as mybir
import concourse.tile as tile
from concourse import bacc
from concourse.bass_utils import run_bass_kernel_spmd

N_CORES = 8
B_FULL = 16384
F = 2048  # contraction (in_features)
H = 96  # heads
B_CORE = B_FULL // N_CORES  # 2048 batch rows per core
P = 128  # partitions
KT = F // P  # 16 k-tiles
BN = 512  # moving free dim per matmul (one PSUM bank of fp32)
BT = B_CORE // BN  # 4 output column tiles per core

_NC_CACHE = {}


_MM_DTYPES = {
    "f32r": (mybir.dt.float32r, np.float32),
    "f32": (mybir.dt.float32, np.float32),
    "f16": (mybir.dt.float16, np.float16),
    "bf16": (mybir.dt.bfloat16, None),  # np dtype resolved lazily (ml_dtypes)
}


def _mm_np_dtype(name):
    dt_mm, dt_np = _MM_DTYPES[name]
    if dt_np is None:
        dt_np = mybir.dt.np(dt_mm)
    return dt_np


def _build(repeat=1, mm="f16", timing_mode=False):
    f32 = mybir.dt.float32
    mm_dt = _MM_DTYPES[mm][0]
    kg = int(os.environ.get("BASS_KG", "8"))
    xbufs = int(os.environ.get("BASS_XBUFS", "2"))
    alt = os.environ.get("BASS_ALT", "0") == "1"

    wfix = os.environ.get("BASS_WFIX", "0") == "1"
    nc = bacc.Bacc("TRN2", target_bir_lowering=False, debug=False, num_devices=N_CORES)
    if not timing_mode:
        # partition-major layout: xTp[p, t, b] = x_shard[b, t*128 + p]
        # -> every DMA group reads one large contiguous run per partition.
        xT = nc.dram_tensor("xTp", [P, KT, B_CORE], mm_dt, kind="ExternalInput")
    wT = nc.dram_tensor("wT", [F, H], mm_dt, kind="ExternalInput")
    wT_lo = (
        nc.dram_tensor("wT_lo", [F, H], mm_dt, kind="ExternalInput") if wfix else None
    )
    bias = nc.dram_tensor("bias", [H, 1], f32, kind="ExternalInput")
    outT = nc.dram_tensor("outT", [H, B_CORE], f32, kind="ExternalOutput")

    with tile.TileContext(nc) as tc:
        if timing_mode:
            # x lives in internal DRAM (garbage contents): identical DMA and
            # compute pattern, but launches don't ship the 16MB/core shard.
            with tc.tile_pool(name="xdram", bufs=1, space="DRAM") as xdram:
                xT = xdram.tile([P, KT, B_CORE], mm_dt, name="xT_int")
        KG = kg  # k-stripes per DMA
        with (
            tc.tile_pool(name="wpool", bufs=1) as wpool,
            tc.tile_pool(name="xpool", bufs=xbufs) as xpool,
            tc.tile_pool(name="pspool", bufs=1, space="PSUM") as pspool,
            tc.tile_pool(name="opool", bufs=2) as opool,
        ):
            wt = wpool.tile([P, KT, H], mm_dt)
            nc.sync.dma_start(wt[:], wT.ap().rearrange("(t p) h -> p t h", p=P))
            wt_lo = None
            if wfix:
                wt_lo = wpool.tile([P, KT, H], mm_dt)
                nc.sync.dma_start(
                    wt_lo[:], wT_lo.ap().rearrange("(t p) h -> p t h", p=P)
                )
            bias_sb = wpool.tile([H, 1], f32)
            nc.sync.dma_start(bias_sb[:], bias[:])

            variant = os.environ.get("BASS_VARIANT", "full")
            taper = os.environ.get("BASS_TAPER", "0") == "1"

            # k-group schedule: uniform KG-sized groups, optionally tapering
            # the last group down (e.g. KG=4 -> [4,4,4,2,1,1]) so the final
            # accumulations (and the output path behind them) expose less.
            groups_env = os.environ.get("BASS_GROUPS", "")
            if groups_env:
                groups = [int(v) for v in groups_env.split(",")]
            else:
                groups = [KG] * (KT // KG)
            if not groups_env and taper and variant == "full" and KG > 1:
                # split the last group into halves: KG=4 -> [2,1,1]
                rem = KG
                groups = [KG] * (KT // KG - 1)
                while rem > 1:
                    h = rem // 2
                    groups.append(h)
                    rem -= h
                groups.append(rem)
            assert sum(groups) == KT, groups

            def emit_mms(ps, k, rhs):
                first, last = k == 0, k == KT - 1
                if not wfix:
                    nc.tensor.matmul(
                        ps[:], lhsT=wt[:, k, :], rhs=rhs, start=first, stop=last
                    )
                else:
                    nc.tensor.matmul(
                        ps[:], lhsT=wt[:, k, :], rhs=rhs, start=first, stop=False
                    )
                    nc.tensor.matmul(
                        ps[:], lhsT=wt_lo[:, k, :], rhs=rhs, start=False, stop=last
                    )

            def emit_out(bt, psums):
                ot = opool.tile([H, BN], f32, tag="ot")
                nc.vector.tensor_scalar_add(ot[:], psums[bt][:], bias_sb[:])
                nc.sync.dma_start(outT[:, bt * BN : (bt + 1) * BN], ot[:])

            def body(_=None):
                n_ps = 8 if variant == "mmnodep" else BT
                psums = [
                    pspool.tile([H, BN], f32, name=f"ps{i}", tag=f"ps{i}")
                    for i in range(n_ps)
                ] if variant != "dmaonly" else [None] * BT
                last_xk = None
                k0 = 0
                for kg_i, glen in enumerate(groups):
                    if variant in ("mm1dma", "mmhalf", "mmnodep") and kg_i > 0:
                        xk = last_xk
                        if xk.shape[1] < glen:
                            k0 += glen
                            continue
                    else:
                        xk = xpool.tile([P, glen, B_CORE], mm_dt, tag="xk")
                        # optionally alternate the two HWDGE rings (SP / ACT)
                        dma_eng = nc.sync if (kg_i % 2 == 0 or not alt) else nc.scalar
                        dma_eng.dma_start(xk[:], xT[:, k0 : k0 + glen, :])
                    last_xk = xk
                    if variant == "dmaonly":
                        k0 += glen
                        continue
                    is_final = k0 + glen == KT
                    n_bt = 2 if variant == "mmhalf" else BT
                    if is_final and variant == "full":
                        # bt-major in the final group: each psum finishes
                        # early and its copy/out-DMA overlaps remaining MMs
                        for bt in range(n_bt):
                            for s in range(glen):
                                k = k0 + s
                                emit_mms(
                                    psums[bt],
                                    k,
                                    xk[:, s, bt * BN : (bt + 1) * BN],
                                )
                            emit_out(bt, psums)
                    else:
                        for s in range(glen):
                            k = k0 + s
                            for bt in range(n_bt):
                                if variant == "mmnodep":
                                    ps = psums[(k * BT + bt) % len(psums)]
                                    nc.tensor.matmul(
                                        ps[:],
                                        lhsT=wt[:, k, :],
                                        rhs=xk[:, s, bt * BN : (bt + 1) * BN],
                                        start=True,
                                        stop=True,
                                    )
                                else:
                                    emit_mms(
                                        psums[bt],
                                        k,
                                        xk[:, s, bt * BN : (bt + 1) * BN],
                                    )
                    k0 += glen
                if variant != "full":
                    for bt in range(BT):
                        ot = opool.tile([H, BN], f32, tag="ot")
                        if variant == "dmaonly":
                            nc.vector.tensor_copy(ot[:], last_xk[0:H, 0, 0:BN])
                        else:
                            src = (
                                psums[bt % 2]
                                if variant == "mmhalf"
                                else psums[bt]
                            )
                            nc.vector.tensor_scalar_add(ot[:], src[:], bias_sb[:])
                        nc.sync.dma_start(outT[:, bt * BN : (bt + 1) * BN], ot[:])

            if repeat == 1:
                body()
            else:
                with tc.For_i(0, repeat, 1):
                    body()

    nc.compile()
    return nc


def _get_nc(repeat, mm, timing_mode=False):
    knobs = tuple(
        os.environ.get(k, "")
        for k in (
            "BASS_KG",
            "BASS_XBUFS",
            "BASS_ALT",
            "BASS_VARIANT",
            "BASS_TAPER",
            "BASS_GROUPS",
            "BASS_WFIX",
        )
    )
    key = (repeat, mm, timing_mode, knobs)
    if key not in _NC_CACHE:
        _NC_CACHE[key] = _build(repeat, mm, timing_mode)
    return _NC_CACHE[key]


def kernel(x, W, b):
    repeat = int(os.environ.get("BASS_KERNEL_REPEAT", "1"))
    mm = os.environ.get("BASS_KERNEL_MM", "f16")
    timing_mode = os.environ.get("BASS_KERNEL_TIMING", "0") == "1"
    nc = _get_nc(repeat, mm, timing_mode)

    np_mm = _mm_np_dtype(mm)
    wfix = os.environ.get("BASS_WFIX", "0") == "1"
    x = np.ascontiguousarray(x, dtype=np.float32)
    W32 = np.asarray(W, dtype=np.float32)
    if wfix:
        import ml_dtypes

        W_hi = W32.astype(ml_dtypes.bfloat16).astype(np.float32)
        W_lo = W32 - W_hi
        wT_host = np.ascontiguousarray(W_hi.T).astype(np_mm)
        wT_lo_host = np.ascontiguousarray(W_lo.T).astype(np_mm)
    else:
        wT_host = np.ascontiguousarray(W32.T).astype(np_mm)
        wT_lo_host = None
    bias_host = np.ascontiguousarray(np.asarray(b, dtype=np.float32).reshape(H, 1))

    in_maps = []
    for i in range(N_CORES):
        shard = x[i * B_CORE : (i + 1) * B_CORE, :]
        m = {
            "wT": wT_host,
            "bias": bias_host,
        }
        if wfix:
            m["wT_lo"] = wT_lo_host
        if not timing_mode:
            # [b, f] -> [f, b] -> [t, p, b] -> [p, t, b] contiguous
            xTp = np.ascontiguousarray(
                shard.T.reshape(KT, P, B_CORE).transpose(1, 0, 2)
            ).astype(np_mm)
            m["xTp"] = xTp
        in_maps.append(m)

    res = run_bass_kernel_spmd(nc, in_maps, core_ids=list(range(N_CORES)))
    out = np.concatenate(
        [np.ascontiguousarray(res.results[i]["outT"].T) for i in range(N_CORES)],
        axis=0,
    )
    return out


# revision 50
# speedup vs baseline: 1.8053x; 1.0786x over previous
"""Trainium2 Bass kernel for nn_CustomMultiHead (96 Linear(2048,1) heads).

Computes out[16384, 96] = x[16384, 2048] @ W.T[2048, 96] + b.

Strategy (data-parallel over batch, 8 cores, 2048 rows each):
  - Host pre-transposes each core's x shard into partition-major
    xTp[p, t, b] (p=partition, t=k-tile, b=batch) so the device kernel
    needs no on-chip transpose (PE matmul contracts along the partition
    dim) and every DMA reads one large contiguous run per partition.
  - Inputs are cast to fp16 on the host: halves HBM traffic (the kernel
    is memory-bound) at ~2.8e-4 scale-relative absmax error; PSUM
    accumulation stays fp32.
  - Per core: out.T[96, 2048] accumulated over 16 k-tiles of 128.
    lhsT = W.T tile [128, 96] (stationary), rhs = xTp tile [128, 512]
    (moving, N=512 = one fp32 PSUM bank); bias added on the PSUM->SBUF
    copy (DVE tensor_scalar_add with a per-partition scalar).
  - x streams through SBUF in 2 double-buffered 4MB DMA groups (8
    k-stripes each) overlapping the matmul stream; in the final group
    the matmuls run bt-major so each PSUM's copy/out-DMA overlaps the
    remaining matmuls.
  - Host transposes/concats the 8 out.T shards back to [16384, 96].

Measured (8-core SPMD, axon): ~30-35us/core vs ~28.6us pure-DMA floor
at the observed ~290GB/s effective HBM rate. fp32r variant (exact fp32
inputs, TF32-class PE path): ~55-60us at 1.2e-4 error; plain fp32:
~80us at 4e-7 (PE-bound, 4 cycles/row). Env knobs (BASS_KERNEL_MM,
BASS_KG, ...) select variants; defaults are the shipped configuration.
"""

import os

import numpy as np

import concourse.mybir as mybir
import concourse.tile as tile
from concourse import bacc
from concourse.bass_utils import run_bass_kernel_spmd

N_CORES = 8
B_FULL = 16384
F = 2048  # contraction (in_features)
H = 96  # heads
B_CORE = B_FULL // N_CORES  # 2048 batch rows per core
P = 128  # partitions
KT = F // P  # 16 k-tiles
BN = 512  # moving free dim per matmul (one PSUM bank of fp32)
BT = B_CORE // BN  # 4 output column tiles per core

_NC_CACHE = {}


_MM_DTYPES = {
    "f32r": (mybir.dt.float32r, np.float32),
    "f32": (mybir.dt.float32, np.float32),
    "f16": (mybir.dt.float16, np.float16),
    "bf16": (mybir.dt.bfloat16, None),  # np dtype resolved lazily (ml_dtypes)
}


def _mm_np_dtype(name):
    dt_mm, dt_np = _MM_DTYPES[name]
    if dt_np is None:
        dt_np = mybir.dt.np(dt_mm)
    return dt_np


def _build(repeat=1, mm="f16", timing_mode=False):
    f32 = mybir.dt.float32
    mm_dt = _MM_DTYPES[mm][0]
    kg = int(os.environ.get("BASS_KG", "8"))
    xbufs = int(os.environ.get("BASS_XBUFS", "2"))
    alt = os.environ.get("BASS_ALT", "0") == "1"

    wfix = os.environ.get("BASS_WFIX", "0") == "1"
    nc = bacc.Bacc("TRN2", target_bir_lowering=False, debug=False, num_devices=N_CORES)
    if not timing_mode:
        # partition-major layout: xTp[p, t, b] = x_shard[b, t*128 + p]
        # -> every DMA group reads one large contiguous run per partition.
        xT = nc.dram_tensor("xTp", [P, KT, B_CORE], mm_dt, kind="ExternalInput")
    wT = nc.dram_tensor("wT", [F, H], mm_dt, kind="ExternalInput")
    wT_lo = (
        nc.dram_tensor("wT_lo", [F, H], mm_dt, kind="ExternalInput") if wfix else None
    )
    bias = nc.dram_tensor("bias", [H, 1], f32, kind="ExternalInput")
    outT = nc.dram_tensor("outT", [H, B_CORE], f32, kind="ExternalOutput")

    with tile.TileContext(nc) as tc:
        if timing_mode:
            # x lives in internal DRAM (garbage contents): identical DMA and
            # compute pattern, but launches don't ship the 16MB/core shard.
            with tc.tile_pool(name="xdram", bufs=1, space="DRAM") as xdram:
                xT = xdram.tile([P, KT, B_CORE], mm_dt, name="xT_int")
        KG = kg  # k-stripes per DMA
        with (
            tc.tile_pool(name="wpool", bufs=1) as wpool,
            tc.tile_pool(name="xpool", bufs=xbufs) as xpool,
            tc.tile_pool(name="pspool", bufs=1, space="PSUM") as pspool,
            tc.tile_pool(name="opool", bufs=2) as opool,
        ):
            wt = wpool.tile([P, KT, H], mm_dt)
            nc.sync.dma_start(wt[:], wT.ap().rearrange("(t p) h -> p t h", p=P))
            wt_lo = None
            if wfix:
                wt_lo = wpool.tile([P, KT, H], mm_dt)
                nc.sync.dma_start(
                    wt_lo[:], wT_lo.ap().rearrange("(t p) h -> p t h", p=P)
                )
            bias_sb = wpool.tile([H, 1], f32)
            nc.sync.dma_start(bias_sb[:], bias[:])

            variant = os.environ.get("BASS_VARIANT", "full")
            taper = os.environ.get("BASS_TAPER", "0") == "1"

            # k-group schedule: uniform KG-sized groups, optionally tapering
            # the last group down (e.g. KG=4 -> [4,4,4,2,1,1]) so the final
            # accumulations (and the output path behind them) expose less.
            groups_env = os.environ.get("BASS_GROUPS", "")
            if groups_env:
                groups = [int(v) for v in groups_env.split(",")]
            else:
                groups = [KG] * (KT // KG)
            if not groups_env and taper and variant == "full" and KG > 1:
                # split the last group into halves: KG=4 -> [2,1,1]
                rem = KG
                groups = [KG] * (KT // KG - 1)
                while rem > 1:
                    h = rem // 2
                    groups.append(h)
                    rem -= h
                groups.append(rem)
            assert sum(groups) == KT, groups

            def emit_mms(ps, k, rhs):
                first, last = k == 0, k == KT - 1
                if not wfix:
                    nc.tensor.matmul(
                        ps[:], lhsT=wt[:, k, :], rhs=rhs, start=first, stop=last
                    )
                else:
                    nc.tensor.matmul(
                        ps[:], lhsT=wt[:, k, :], rhs=rhs, start=first, stop=False
                    )
                    nc.tensor.matmul(
                        ps[:], lhsT=wt_lo[:, k, :], rhs=rhs, start=False, stop=last
                    )

            def emit_out(bt, psums):
                ot = opool.tile([H, BN], f32, tag="ot")
                nc.vector.tensor_scalar_add(ot[:], psums[bt][:], bias_sb[:])
                nc.sync.dma_start(outT[:, bt * BN : (bt + 1) * BN], ot[:])

            def body(_=None):
                n_ps = 8 if variant == "mmnodep" else BT
                psums = [
                    pspool.tile([H, BN], f32, name=f"ps{i}", tag=f"ps{i}")
                    for i in range(n_ps)
                ] if variant != "dmaonly" else [None] * BT
                last_xk = None
                k0 = 0
                for kg_i, glen in enumerate(groups):
                    if variant in ("mm1dma", "mmhalf", "mmnodep") and kg_i > 0:
                        xk = last_xk
                        if xk.shape[1] < glen:
                            k0 += glen
                            continue
                    else:
                        xk = xpool.tile([P, glen, B_CORE], mm_dt, tag="xk")
                        # optionally alternate the two HWDGE rings (SP / ACT)
                        dma_eng = nc.sync if (kg_i % 2 == 0 or not alt) else nc.scalar
                        dma_eng.dma_start(xk[:], xT[:, k0 : k0 + glen, :])
                    last_xk = xk
                    if variant == "dmaonly":
                        k0 += glen
                        continue
                    is_final = k0 + glen == KT
                    n_bt = 2 if variant == "mmhalf" else BT
                    if is_final and variant == "full":
                        # bt-major in the final group: each psum finishes
                        # early and its copy/out-DMA overlaps remaining MMs
                        for bt in range(n_bt):
                            for s in range(glen):
                                k = k0 + s
                                emit_mms(
                                    psums[bt],
                                    k,
                                    xk[:, s, bt * BN : (bt + 1) * BN],
                                )
                            emit_out(bt, psums)
                    else:
                        for s in range(glen):
                            k = k0 + s
                            for bt in range(n_bt):
                                if variant == "mmnodep":
                                    ps = psums[(k * BT + bt) % len(psums)]
                                    nc.tensor.matmul(
                                        ps[:],
                                        lhsT=wt[:, k, :],
                                        rhs=xk[:, s, bt * BN : (bt + 1) * BN],
                                        start=True,
                                        stop=True,
                                    )
                                else:
                                    emit_mms(
                                        psums[bt],
                                        k,
                                        xk[:, s, bt * BN : (bt + 1) * BN],
                                    )
                    k0 += glen
                if variant != "full":
                    for bt in range(BT):
                        ot = opool.tile([H, BN], f32, tag="ot")
                        if variant == "dmaonly":
                            nc.vector.tensor_copy(ot[:], last_xk[0:H, 0, 0:BN])
                        else:
                            src = (
                                psums[bt % 2]
                                if variant == "mmhalf"
                                else psums[bt]
                            )
                            nc.vector.tensor_scalar_add(ot[:], src[:], bias_sb[:])
                        nc.sync.dma_start(outT[:, bt * BN : (bt + 1) * BN], ot[:])

            if repeat == 1:
                body()
            else:
                with tc.For_i(0, repeat, 1):
                    body()

    nc.compile()
    return nc


def _get_nc(repeat, mm, timing_mode=False):
    knobs = tuple(
        os.environ.get(k, "")
        for k in (
            "BASS_KG",
            "BASS_XBUFS",
            "BASS_ALT",
            "BASS_VARIANT",
            "BASS_TAPER",
            "BASS_GROUPS",
            "BASS_WFIX",
        )
    )
    key = (repeat, mm, timing_mode, knobs)
    if key not in _NC_CACHE:
        _NC_CACHE[key] = _build(repeat, mm, timing_mode)
    return _NC_CACHE[key]


def kernel(x, W, b):
    repeat = int(os.environ.get("BASS_KERNEL_REPEAT", "1"))
    mm = os.environ.get("BASS_KERNEL_MM", "f16")
    timing_mode = os.environ.get("BASS_KERNEL_TIMING", "0") == "1"
    nc = _get_nc(repeat, mm, timing_mode)

    np_mm = _mm_np_dtype(mm)
    wfix = os.environ.get("BASS_WFIX", "0") == "1"
    x = np.ascontiguousarray(x, dtype=np.float32)
    W32 = np.asarray(W, dtype=np.float32)
    if wfix:
        import ml_dtypes

        W_hi = W32.astype(ml_dtypes.bfloat16).astype(np.float32)
        W_lo = W32 - W_hi
        wT_host = np.ascontiguousarray(W_hi.T).astype(np_mm)
        wT_lo_host = np.ascontiguousarray(W_lo.T).astype(np_mm)
    else:
        wT_host = np.ascontiguousarray(W32.T).astype(np_mm)
        wT_lo_host = None
    bias_host = np.ascontiguousarray(np.asarray(b, dtype=np.float32).reshape(H, 1))

    in_maps = []
    for i in range(N_CORES):
        shard = x[i * B_CORE : (i + 1) * B_CORE, :]
        m = {
            "wT": wT_host,
            "bias": bias_host,
        }
        if wfix:
            m["wT_lo"] = wT_lo_host
        if not timing_mode:
            # [b, f] -> [f, b] -> [t, p, b] -> [p, t, b] contiguous
            # (cast first so the big gather copy moves half the bytes)
            xTp = np.ascontiguousarray(
                shard.astype(np_mm).T.reshape(KT, P, B_CORE).transpose(1, 0, 2)
            )
            m["xTp"] = xTp
        in_maps.append(m)

    res = run_bass_kernel_spmd(nc, in_maps, core_ids=list(range(N_CORES)))
    out = np.concatenate(
        [np.ascontiguousarray(res.results[i]["outT"].T) for i in range(N_CORES)],
        axis=0,
    )
    return out


# revision 51
# speedup vs baseline: 1.8954x; 1.0499x over previous
"""Trainium2 Bass kernel for nn_CustomMultiHead (96 Linear(2048,1) heads).

Computes out[16384, 96] = x[16384, 2048] @ W.T[2048, 96] + b.

Strategy (data-parallel over batch, 8 cores, 2048 rows each):
  - Host pre-transposes each core's x shard into partition-major
    xTp[p, t, b] (p=partition, t=k-tile, b=batch) so the device kernel
    needs no on-chip transpose (PE matmul contracts along the partition
    dim) and every DMA reads one large contiguous run per partition.
  - Inputs are cast to fp16 on the host: halves HBM traffic (the kernel
    is memory-bound) at ~2.8e-4 scale-relative absmax error; PSUM
    accumulation stays fp32.
  - Per core: out.T[96, 2048] accumulated over 16 k-tiles of 128.
    lhsT = W.T tile [128, 96] (stationary), rhs = xTp tile [128, 512]
    (moving, N=512 = one fp32 PSUM bank); bias added on the PSUM->SBUF
    copy (DVE tensor_scalar_add with a per-partition scalar).
  - x streams through SBUF in 2 double-buffered 4MB DMA groups (8
    k-stripes each) overlapping the matmul stream; in the final group
    the matmuls run bt-major so each PSUM's copy/out-DMA overlaps the
    remaining matmuls.
  - Host transposes/concats the 8 out.T shards back to [16384, 96].

Measured (8-core SPMD, axon): ~30-35us/core vs ~28.6us pure-DMA floor
at the observed ~290GB/s effective HBM rate. fp32r variant (exact fp32
inputs, TF32-class PE path): ~55-60us at 1.2e-4 error; plain fp32:
~80us at 4e-7 (PE-bound, 4 cycles/row). Env knobs (BASS_KERNEL_MM,
BASS_KG, ...) select variants; defaults are the shipped configuration.
"""

import os

import numpy as np

import concourse.mybir as mybir
import concourse.tile as tile
from concourse import bacc
from concourse.bass_utils import run_bass_kernel_spmd

N_CORES = 8
B_FULL = 16384
F = 2048  # contraction (in_features)
H = 96  # heads
B_CORE = B_FULL // N_CORES  # 2048 batch rows per core
P = 128  # partitions
KT = F // P  # 16 k-tiles
BN = 512  # moving free dim per matmul (one PSUM bank of fp32)
BT = B_CORE // BN  # 4 output column tiles per core

_NC_CACHE = {}


_MM_DTYPES = {
    "f32r": (mybir.dt.float32r, np.float32),
    "f32": (mybir.dt.float32, np.float32),
    "f16": (mybir.dt.float16, np.float16),
    "bf16": (mybir.dt.bfloat16, None),  # np dtype resolved lazily (ml_dtypes)
}


def _mm_np_dtype(name):
    dt_mm, dt_np = _MM_DTYPES[name]
    if dt_np is None:
        dt_np = mybir.dt.np(dt_mm)
    return dt_np


def _build(repeat=1, mm="f16", timing_mode=False):
    f32 = mybir.dt.float32
    mm_dt = _MM_DTYPES[mm][0]
    kg = int(os.environ.get("BASS_KG", "8"))
    xbufs = int(os.environ.get("BASS_XBUFS", "2"))
    alt = os.environ.get("BASS_ALT", "0") == "1"

    wfix = os.environ.get("BASS_WFIX", "0") == "1"
    nc = bacc.Bacc("TRN2", target_bir_lowering=False, debug=False, num_devices=N_CORES)
    if not timing_mode:
        # partition-major layout: xTp[p, t, b] = x_shard[b, t*128 + p]
        # -> every DMA group reads one large contiguous run per partition.
        xT = nc.dram_tensor("xTp", [P, KT, B_CORE], mm_dt, kind="ExternalInput")
    wT = nc.dram_tensor("wT", [F, H], mm_dt, kind="ExternalInput")
    wT_lo = (
        nc.dram_tensor("wT_lo", [F, H], mm_dt, kind="ExternalInput") if wfix else None
    )
    bias = nc.dram_tensor("bias", [H, 1], f32, kind="ExternalInput")
    outT = nc.dram_tensor("outT", [H, B_CORE], f32, kind="ExternalOutput")

    with tile.TileContext(nc) as tc:
        if timing_mode:
            # x lives in internal DRAM (garbage contents): identical DMA and
            # compute pattern, but launches don't ship the 16MB/core shard.
            with tc.tile_pool(name="xdram", bufs=1, space="DRAM") as xdram:
                xT = xdram.tile([P, KT, B_CORE], mm_dt, name="xT_int")
        KG = kg  # k-stripes per DMA
        with (
            tc.tile_pool(name="wpool", bufs=1) as wpool,
            tc.tile_pool(name="xpool", bufs=xbufs) as xpool,
            tc.tile_pool(name="pspool", bufs=1, space="PSUM") as pspool,
            tc.tile_pool(name="opool", bufs=2) as opool,
        ):
            # W/bias ride the ACT HWDGE ring so the x-stream DMAs (SP ring)
            # start immediately in the single-shot run.
            wt = wpool.tile([P, KT, H], mm_dt)
            nc.scalar.dma_start(wt[:], wT.ap().rearrange("(t p) h -> p t h", p=P))
            wt_lo = None
            if wfix:
                wt_lo = wpool.tile([P, KT, H], mm_dt)
                nc.scalar.dma_start(
                    wt_lo[:], wT_lo.ap().rearrange("(t p) h -> p t h", p=P)
                )
            bias_sb = wpool.tile([H, 1], f32)
            nc.scalar.dma_start(bias_sb[:], bias[:])

            variant = os.environ.get("BASS_VARIANT", "full")
            taper = os.environ.get("BASS_TAPER", "0") == "1"

            # k-group schedule: uniform KG-sized groups, optionally tapering
            # the last group down (e.g. KG=4 -> [4,4,4,2,1,1]) so the final
            # accumulations (and the output path behind them) expose less.
            groups_env = os.environ.get("BASS_GROUPS", "")
            if groups_env:
                groups = [int(v) for v in groups_env.split(",")]
            else:
                groups = [KG] * (KT // KG)
            if not groups_env and taper and variant == "full" and KG > 1:
                # split the last group into halves: KG=4 -> [2,1,1]
                rem = KG
                groups = [KG] * (KT // KG - 1)
                while rem > 1:
                    h = rem // 2
                    groups.append(h)
                    rem -= h
                groups.append(rem)
            assert sum(groups) == KT, groups

            def emit_mms(ps, k, rhs):
                first, last = k == 0, k == KT - 1
                if not wfix:
                    nc.tensor.matmul(
                        ps[:], lhsT=wt[:, k, :], rhs=rhs, start=first, stop=last
                    )
                else:
                    nc.tensor.matmul(
                        ps[:], lhsT=wt[:, k, :], rhs=rhs, start=first, stop=False
                    )
                    nc.tensor.matmul(
                        ps[:], lhsT=wt_lo[:, k, :], rhs=rhs, start=False, stop=last
                    )

            def emit_out(bt, psums):
                ot = opool.tile([H, BN], f32, tag="ot")
                nc.vector.tensor_scalar_add(ot[:], psums[bt][:], bias_sb[:])
                nc.sync.dma_start(outT[:, bt * BN : (bt + 1) * BN], ot[:])

            def body(_=None):
                n_ps = 8 if variant == "mmnodep" else BT
                psums = [
                    pspool.tile([H, BN], f32, name=f"ps{i}", tag=f"ps{i}")
                    for i in range(n_ps)
                ] if variant != "dmaonly" else [None] * BT
                last_xk = None
                k0 = 0
                for kg_i, glen in enumerate(groups):
                    if variant in ("mm1dma", "mmhalf", "mmnodep") and kg_i > 0:
                        xk = last_xk
                        if xk.shape[1] < glen:
                            k0 += glen
                            continue
                    else:
                        xk = xpool.tile([P, glen, B_CORE], mm_dt, tag="xk")
                        # optionally alternate the two HWDGE rings (SP / ACT)
                        dma_eng = nc.sync if (kg_i % 2 == 0 or not alt) else nc.scalar
                        dma_eng.dma_start(xk[:], xT[:, k0 : k0 + glen, :])
                    last_xk = xk
                    if variant == "dmaonly":
                        k0 += glen
                        continue
                    is_final = k0 + glen == KT
                    n_bt = 2 if variant == "mmhalf" else BT
                    if is_final and variant == "full":
                        # bt-major in the final group: each psum finishes
                        # early and its copy/out-DMA overlaps remaining MMs
                        for bt in range(n_bt):
                            for s in range(glen):
                                k = k0 + s
                                emit_mms(
                                    psums[bt],
                                    k,
                                    xk[:, s, bt * BN : (bt + 1) * BN],
                                )
                            emit_out(bt, psums)
                    else:
                        for s in range(glen):
                            k = k0 + s
                            for bt in range(n_bt):
                                if variant == "mmnodep":
                                    ps = psums[(k * BT + bt) % len(psums)]
                                    nc.tensor.matmul(
                                        ps[:],
                                        lhsT=wt[:, k, :],
                                        rhs=xk[:, s, bt * BN : (bt + 1) * BN],
                                        start=True,
                                        stop=True,
                                    )
                                else:
                                    emit_mms(
                                        psums[bt],
                                        k,
                                        xk[:, s, bt * BN : (bt + 1) * BN],
                                    )
                    k0 += glen
                if variant != "full":
                    for bt in range(BT):
                        ot = opool.tile([H, BN], f32, tag="ot")
                        if variant == "dmaonly":
                            nc.vector.tensor_copy(ot[:], last_xk[0:H, 0, 0:BN])
                        else:
                            src = (
                                psums[bt % 2]
                                if variant == "mmhalf"
                                else psums[bt]
                            )
                            nc.vector.tensor_scalar_add(ot[:], src[:], bias_sb[:])
                        nc.sync.dma_start(outT[:, bt * BN : (bt + 1) * BN], ot[:])

            if repeat == 1:
                body()
            else:
                with tc.For_i(0, repeat, 1):
                    body()

    nc.compile()
    return nc


def _get_nc(repeat, mm, timing_mode=False):
    knobs = tuple(
        os.environ.get(k, "")
        for k in (
            "BASS_KG",
            "BASS_XBUFS",
            "BASS_ALT",
            "BASS_VARIANT",
            "BASS_TAPER",
            "BASS_GROUPS",
            "BASS_WFIX",
        )
    )
    key = (repeat, mm, timing_mode, knobs)
    if key not in _NC_CACHE:
        _NC_CACHE[key] = _build(repeat, mm, timing_mode)
    return _NC_CACHE[key]


def kernel(x, W, b):
    repeat = int(os.environ.get("BASS_KERNEL_REPEAT", "1"))
    mm = os.environ.get("BASS_KERNEL_MM", "f16")
    timing_mode = os.environ.get("BASS_KERNEL_TIMING", "0") == "1"
    nc = _get_nc(repeat, mm, timing_mode)

    np_mm = _mm_np_dtype(mm)
    wfix = os.environ.get("BASS_WFIX", "0") == "1"
    x = np.ascontiguousarray(x, dtype=np.float32)
    W32 = np.asarray(W, dtype=np.float32)
    if wfix:
        import ml_dtypes

        W_hi = W32.astype(ml_dtypes.bfloat16).astype(np.float32)
        W_lo = W32 - W_hi
        wT_host = np.ascontiguousarray(W_hi.T).astype(np_mm)
        wT_lo_host = np.ascontiguousarray(W_lo.T).astype(np_mm)
    else:
        wT_host = np.ascontiguousarray(W32.T).astype(np_mm)
        wT_lo_host = None
    bias_host = np.ascontiguousarray(np.asarray(b, dtype=np.float32).reshape(H, 1))

    in_maps = []
    for i in range(N_CORES):
        shard = x[i * B_CORE : (i + 1) * B_CORE, :]
        m = {
            "wT": wT_host,
            "bias": bias_host,
        }
        if wfix:
            m["wT_lo"] = wT_lo_host
        if not timing_mode:
            # [b, f] -> [f, b] -> [t, p, b] -> [p, t, b] contiguous
            # (cast first so the big gather copy moves half the bytes)
            xTp = np.ascontiguousarray(
                shard.astype(np_mm).T.reshape(KT, P, B_CORE).transpose(1, 0, 2)
            )
            m["xTp"] = xTp
        in_maps.append(m)

    res = run_bass_kernel_spmd(nc, in_maps, core_ids=list(range(N_CORES)))
    out = np.concatenate(
        [np.ascontiguousarray(res.results[i]["outT"].T) for i in range(N_CORES)],
        axis=0,
    )
    return out
